# revision 25
# baseline (speedup 1.0000x reference)
"""Distributed memory-shard scale kernel for Trainium2 (8 NeuronCores).

Computes out[b, s, d] = x[b, s, d] * shards[shard_map[d], d] for
x: [4, 4096, 4096] f32, shards: [8, 4096] f32, shard_map: [4096] int.

Strategy: data-parallel over the flattened (batch*seq) rows — each of the
8 cores owns a contiguous 2048-row slice of x and replicates the tiny
shards/shard_map inputs. On device each core:
  1. builds w[d] = shards[shard_map[d], d] with masked multiply-accumulate
     over the 8 shard rows, laid out 32 dims per partition so all 128 DVE
     lanes work,
  2. flattens w into a DRAM scratch row and replicates it to all 128
     partitions with K=1 outer-product matmuls (PE + DVE only),
  3. streams its x slice through SBUF in [128, 4096] tiles, multiplying by
     the replicated weight row and writing back out.
"""

import numpy as np

import bass_rust as _bass_rust
import concourse.bass as bass
import concourse.tile as tile
from concourse import mybir
from concourse.bass_utils import run_bass_kernel_spmd

N_CORES = 8
BATCH, SEQ, DIM = 4, 4096, 4096
NUM_SHARDS = 8
ROWS_TOTAL = BATCH * SEQ               # 16384
ROWS_PER_CORE = ROWS_TOTAL // N_CORES  # 2048
P = 128                                # SBUF partitions
DPP = DIM // P                         # dims per partition in w layout (32)
N_TILES = ROWS_PER_CORE // P           # 16
AUX_ONES = (1 + NUM_SHARDS) * DPP      # col where the ones row starts (288)
AUX_W = AUX_ONES + P                   # aux free width (416)

TRACE = False       # set True (e.g. from test.py) to capture an NTFF profile
LAST_RESULT = None  # BassKernelResults of the most recent kernel() call

_cached_nc = None


def _build_program() -> bass.Bass:
    f32 = mybir.dt.float32
    nc = bass.Bass()
    x_in = nc.dram_tensor("x", [ROWS_PER_CORE, DIM], f32, kind="ExternalInput")
    # aux packs shard_map, shards, and a ones row in one tensor (single DMA
    # → single sync wait on the first consumer):
    #   aux[p, 0:DPP]         = shard_map[p*DPP : (p+1)*DPP]  (as f32)
    #   aux[p, (1+s)*DPP + j] = shards[s, p*DPP + j]
    #   aux[p, AUX_ONES:]     = 1.0  (matmul lhsT for the broadcast)
    aux_in = nc.dram_tensor("aux", [P, AUX_W], f32, kind="ExternalInput")
    out = nc.dram_tensor("out", [ROWS_PER_CORE, DIM], f32,
                         kind="ExternalOutput")
    # scratch row: w (4096) followed by the ones row (128)
    w_scratch = nc.dram_tensor("w_scratch", [DIM + P], f32)

    with tile.TileContext(nc) as tc:
        with tc.tile_pool(name="const", bufs=1) as cpool, \
             tc.tile_pool(name="xp", bufs=10) as xpool:
            # --- one-time: w[d] = shards[shard_map[d], d], 32 dims/partition
            # (the whole w chain rides the ACT HWDGE ring — idle until the
            # first store, which transitively depends on w anyway — so it
            # never queues behind the streaming x loads on the SP ring)
            auxt = cpool.tile([P, AUX_ONES], f32)
            nc.scalar.dma_start(auxt[:], aux_in[:, 0:AUX_ONES])
            # ones row into the scratch tail: DRAM→DRAM, zero dependencies
            nc.scalar.dma_start(w_scratch[DIM:DIM + P],
                                aux_in[0:1, AUX_ONES:AUX_W])
            mf = auxt[:, 0:DPP]
            wacc = cpool.tile([P, DPP], f32)
            tmp = cpool.tile([P, DPP], f32)
            nc.vector.memset(wacc[:], 0.0)
            for s in range(NUM_SHARDS):
                # tmp = (shard_map == s) * shards[s, :]
                nc.vector.scalar_tensor_tensor(
                    out=tmp[:], in0=mf, scalar=float(s),
                    in1=auxt[:, (1 + s) * DPP:(2 + s) * DPP],
                    op0=mybir.AluOpType.is_equal, op1=mybir.AluOpType.mult)
                nc.vector.tensor_add(wacc[:], wacc[:], tmp[:])

            # --- flatten w to one partition row (16KB out + readback on
            # the ACT ring), then replicate to all 128 partitions with K=1
            # outer-product matmuls ones[1,128].T @ wrow[1,512] →
            # PSUM[128,512]. PE+DVE only — no 2MB broadcast DMA, and each
            # matmul carries exactly one sync wait (the readback).
            w128 = cpool.tile([P, DIM], f32)
            wrow = cpool.tile([1, DIM + P], f32)
            nc.scalar.dma_start(w_scratch[0:DIM], wacc[:])
            nc.scalar.dma_start(wrow[:], w_scratch[:])
            ones = wrow[0:1, DIM:DIM + P]
            MMF = 512  # one PSUM bank per matmul
            with tc.tile_pool(name="ps", bufs=8, space="PSUM") as ppool:
                for k in range(DIM // MMF):
                    mm = ppool.tile([P, MMF], f32)
                    nc.tensor.matmul(mm[:], ones,
                                     wrow[0:1, k * MMF:(k + 1) * MMF],
                                     start=True, stop=True)
                    nc.vector.tensor_copy(w128[:, k * MMF:(k + 1) * MMF],
                                          mm[:])

            # --- stream x through SBUF, scaling by w ---
            for i in range(N_TILES):
                xt = xpool.tile([P, DIM], f32)
                nc.sync.dma_start(xt[:], x_in[i * P:(i + 1) * P, :])
                if i < N_TILES - 1:
                    nc.vector.tensor_mul(xt[:], xt[:], w128[:])
                    nc.scalar.dma_start(out[i * P:(i + 1) * P, :], xt[:])
                else:
                    # split the last tile so the final mul→store dependency
                    # chain is half as long
                    h = DIM // 2
                    nc.vector.tensor_mul(xt[:, 0:h], xt[:, 0:h],
                                         w128[:, 0:h])
                    nc.scalar.dma_start(out[i * P:(i + 1) * P, 0:h],
                                        xt[:, 0:h])
                    nc.vector.tensor_mul(xt[:, h:], xt[:, h:], w128[:, h:])
                    nc.scalar.dma_start(out[i * P:(i + 1) * P, h:],
                                        xt[:, h:])
    # TRN2 allows one sync wait per instruction; split multi-wait
    # instructions the way bacc's compile pipeline does.
    _bass_rust.generate_event_semaphores(nc)
    return nc


def _marshal(shards: np.ndarray, shard_map: np.ndarray):
    sh = np.asarray(shards, dtype=np.float32)
    aux = np.empty((P, AUX_W), dtype=np.float32)
    aux[:, 0:DPP] = np.asarray(shard_map).astype(np.float32).reshape(P, DPP)
    # aux[p, (1+s)*DPP + j] = shards[s, p*DPP + j]
    aux[:, DPP:AUX_ONES] = sh.reshape(NUM_SHARDS, P, DPP).transpose(
        1, 0, 2).reshape(P, NUM_SHARDS * DPP)
    aux[:, AUX_ONES:] = 1.0
    return aux


def kernel(x, shards, shard_map):
    global _cached_nc, LAST_RESULT
    if _cached_nc is None:
        _cached_nc = _build_program()
    nc = _cached_nc

    x2 = np.asarray(x, dtype=np.float32).reshape(ROWS_TOTAL, DIM)
    aux = _marshal(shards, shard_map)

    in_maps = [
        {"x": x2[c * ROWS_PER_CORE:(c + 1) * ROWS_PER_CORE], "aux": aux}
        for c in range(N_CORES)
    ]
    res = run_bass_kernel_spmd(nc, in_maps, core_ids=list(range(N_CORES)),
                               trace=TRACE)
    LAST_RESULT = res
    return np.concatenate([r["out"] for r in res.results],
                          axis=0).reshape(BATCH, SEQ, DIM)


# revision 26
# speedup vs baseline: 1.2279x; 1.2279x over previous
"""Distributed memory-shard scale kernel for Trainium2 (8 NeuronCores).

Computes out[b, s, d] = x[b, s, d] * shards[shard_map[d], d] for
x: [4, 4096, 4096] f32, shards: [8, 4096] f32, shard_map: [4096] int.

Strategy: data-parallel over the flattened (batch*seq) rows — each of the
8 cores owns a contiguous 2048-row slice of x and replicates the tiny
shards/shard_map inputs. On device each core:
  1. builds w[d] = shards[shard_map[d], d] with masked multiply-accumulate
     over the 8 shard rows, 256 dims per partition on 16 partitions,
  2. flattens w into a DRAM scratch row and replicates it to all 128
     partitions with K=1 outer-product matmuls (PE + DVE only),
  3. streams its x slice through SBUF in [128, 4096] tiles, multiplying by
     the replicated weight row and writing back out.

The whole w chain (aux load, scratch write, readback) rides SWDGE
(gpsimd) queues: the Tile scheduler multiplexes all HWDGE completions
onto 8 shared semaphore lanes, so a HWDGE wait in the w chain would
serialize behind ~2 queued 2MB x-loads per hop; the SWDGE lanes carry
nothing else, keeping the chain latency at a few microseconds.
"""

import numpy as np

import bass_rust as _bass_rust
import concourse.bass as bass
import concourse.tile as tile
from concourse import mybir
from concourse.bass_utils import run_bass_kernel_spmd

N_CORES = 8
BATCH, SEQ, DIM = 4, 4096, 4096
NUM_SHARDS = 8
ROWS_TOTAL = BATCH * SEQ               # 16384
ROWS_PER_CORE = ROWS_TOTAL // N_CORES  # 2048
P = 128                                # SBUF partitions
N_TILES = ROWS_PER_CORE // P           # 16
WP = 16                                # partitions used by the w build
DPW = DIM // WP                        # dims per partition in w build (256)
AUX_ONES = (1 + NUM_SHARDS) * DPW      # col where the ones row starts (2304)
AUX_W = AUX_ONES + P                   # aux free width (2432)

TRACE = False       # set True (e.g. from test.py) to capture an NTFF profile
LAST_RESULT = None  # BassKernelResults of the most recent kernel() call

_cached_nc = None


def _build_program() -> bass.Bass:
    f32 = mybir.dt.float32
    nc = bass.Bass()
    x_in = nc.dram_tensor("x", [ROWS_PER_CORE, DIM], f32, kind="ExternalInput")
    # aux packs shard_map, shards, and a ones row into one [16, 2432]
    # tensor (16 DMA descriptors on SWDGE):
    #   aux[p, 0:DPW]         = shard_map[p*DPW : (p+1)*DPW]  (as f32)
    #   aux[p, (1+s)*DPW + j] = shards[s, p*DPW + j]
    #   aux[0, AUX_ONES:]     = 1.0  (matmul lhsT for the broadcast)
    aux_in = nc.dram_tensor("aux", [WP, AUX_W], f32, kind="ExternalInput")
    out = nc.dram_tensor("out", [ROWS_PER_CORE, DIM], f32,
                         kind="ExternalOutput")
    # scratch row: w (4096) followed by the ones row (128)
    w_scratch = nc.dram_tensor("w_scratch", [DIM + P], f32)

    with tile.TileContext(nc) as tc:
        with tc.tile_pool(name="const", bufs=1) as cpool, \
             tc.tile_pool(name="xp", bufs=10) as xpool:
            # ones row into the scratch tail: DRAM→DRAM, zero dependencies
            nc.gpsimd.dma_start(w_scratch[DIM:DIM + P],
                                aux_in[0:1, AUX_ONES:AUX_W])
            # --- one-time: w[d] = shards[shard_map[d], d], [16, 256] ---
            auxt = cpool.tile([WP, AUX_ONES], f32)
            nc.gpsimd.dma_start(auxt[:], aux_in[:, 0:AUX_ONES])
            mf = auxt[:, 0:DPW]
            wacc = cpool.tile([WP, DPW], f32)
            tmp = cpool.tile([WP, DPW], f32)
            nc.vector.memset(wacc[:], 0.0)
            for s in range(NUM_SHARDS):
                # tmp = (shard_map == s) * shards[s, :]
                nc.vector.scalar_tensor_tensor(
                    out=tmp[:], in0=mf, scalar=float(s),
                    in1=auxt[:, (1 + s) * DPW:(2 + s) * DPW],
                    op0=mybir.AluOpType.is_equal, op1=mybir.AluOpType.mult)
                nc.vector.tensor_add(wacc[:], wacc[:], tmp[:])

            # --- flatten w to one row of DRAM scratch (16 descriptors),
            # read it back contiguously, then replicate to all 128
            # partitions with K=1 outer-product matmuls ones[1,128].T @
            # wrow[1,512] → PSUM[128,512]; PE+DVE only.
            w128 = cpool.tile([P, DIM], f32)
            wrow = cpool.tile([1, DIM + P], f32)
            nc.gpsimd.dma_start(w_scratch[0:DIM], wacc[:])
            nc.gpsimd.dma_start(wrow[:], w_scratch[:])
            ones = wrow[0:1, DIM:DIM + P]
            MMF = 512  # one PSUM bank per matmul
            with tc.tile_pool(name="ps", bufs=8, space="PSUM") as ppool:
                for k in range(DIM // MMF):
                    mm = ppool.tile([P, MMF], f32)
                    nc.tensor.matmul(mm[:], ones,
                                     wrow[0:1, k * MMF:(k + 1) * MMF],
                                     start=True, stop=True)
                    nc.vector.tensor_copy(w128[:, k * MMF:(k + 1) * MMF],
                                          mm[:])

            # --- stream x through SBUF, scaling by w ---
            for i in range(N_TILES):
                xt = xpool.tile([P, DIM], f32)
                nc.sync.dma_start(xt[:], x_in[i * P:(i + 1) * P, :])
                if i < N_TILES - 1:
                    nc.vector.tensor_mul(xt[:], xt[:], w128[:])
                    nc.scalar.dma_start(out[i * P:(i + 1) * P, :], xt[:])
                else:
                    # split the last tile so the final mul→store dependency
                    # chain is half as long
                    h = DIM // 2
                    nc.vector.tensor_mul(xt[:, 0:h], xt[:, 0:h],
                                         w128[:, 0:h])
                    nc.scalar.dma_start(out[i * P:(i + 1) * P, 0:h],
                                        xt[:, 0:h])
                    nc.vector.tensor_mul(xt[:, h:], xt[:, h:], w128[:, h:])
                    nc.scalar.dma_start(out[i * P:(i + 1) * P, h:],
                                        xt[:, h:])
    # TRN2 allows one sync wait per instruction; split multi-wait
    # instructions the way bacc's compile pipeline does.
    _bass_rust.generate_event_semaphores(nc)
    return nc


def _marshal(shards: np.ndarray, shard_map: np.ndarray):
    sh = np.asarray(shards, dtype=np.float32)
    aux = np.zeros((WP, AUX_W), dtype=np.float32)
    aux[:, 0:DPW] = np.asarray(shard_map).astype(np.float32).reshape(WP, DPW)
    # aux[p, (1+s)*DPW + j] = shards[s, p*DPW + j]
    aux[:, DPW:AUX_ONES] = sh.reshape(NUM_SHARDS, WP, DPW).transpose(
        1, 0, 2).reshape(WP, NUM_SHARDS * DPW)
    aux[0, AUX_ONES:] = 1.0
    return aux


def kernel(x, shards, shard_map):
    global _cached_nc, LAST_RESULT
    if _cached_nc is None:
        _cached_nc = _build_program()
    nc = _cached_nc

    x2 = np.asarray(x, dtype=np.float32).reshape(ROWS_TOTAL, DIM)
    aux = _marshal(shards, shard_map)

    in_maps = [
        {"x": x2[c * ROWS_PER_CORE:(c + 1) * ROWS_PER_CORE], "aux": aux}
        for c in range(N_CORES)
    ]
    res = run_bass_kernel_spmd(nc, in_maps, core_ids=list(range(N_CORES)),
                               trace=TRACE)
    LAST_RESULT = res
    return np.concatenate([r["out"] for r in res.results],
                          axis=0).reshape(BATCH, SEQ, DIM)
